# revision 13
# baseline (speedup 1.0000x reference)
"""Trainium2 Bass kernel for a 3-layer tanh RNN (B=256, T=16384, H=16).

Strategy
--------
Data-parallel: batch 256 is split across 8 NeuronCores (32 per core). The
three RNN layers are fused into ONE recurrence via pipeline skew: at
iteration s the combined state vector is

    v_s = [h0 (16 rows); h1 (16 rows); hp (1 row); x_s (1 row)]   (34 rows)

laid out h-on-partitions, batch-on-free-dim [34, 32]. One 34x33 matmul
computes all pre-activations (layer0 from h0+x, layer1 from h0+h1, layerp
from h1+hp -- using values one step stale, which is exactly the layer skew),
then one scalar-engine tanh with per-partition bias produces v_{s+1}
rows 0..32.  Row 33 (raw x; min-max normalization is folded into the
matmul column and bias) is staged ahead by DMA from a host-transposed
x.  Outputs y[t] = row 32 of v_{t+3}; they are DMA'd out in 256-state
blocks from the rotating slot buffer.

Per step the critical path is exactly: matmul -> tanh -> matmul ...
(one TensorE instruction + one ScalarE instruction, with waits attached
directly to the instructions).
"""

import sys

sys.path.insert(0, "/opt/trn_rl_repo")

import numpy as np

# ---- problem constants (hardcoded; kernel.py must be self-contained) ----
B, T, IN, H, OUT = 256, 16384, 1, 16, 1
NCORES = 8
BL = B // NCORES  # 32 batch elements per core
NSTATE = 2 * H + OUT  # 33 state rows produced per step
K = NSTATE + 1  # 34 = contraction dim (state + x row)
S = T + 2  # 16386 iterations (pipeline skew of 2)
R = 512  # rotating state slots in SBUF
BLK = 256  # states per staging/output block
NBLK = (S + 1 + BLK - 1) // BLK  # 65 blocks cover states v_0..v_{NBLK*BLK-1}
XROWS = NBLK * BLK  # 16640 padded x/y rows

_CACHE = {}


def _build_nc(s_steps=S, nblk=NBLK, half=False):
    import concourse.bass as bass
    import concourse.mybir as mybir

    f32 = mybir.dt.float32
    dt = mybir.dt.float16 if half else f32
    xrows = nblk * BLK

    nc = bass.Bass()
    wT_d = nc.dram_tensor("wT", [K, NSTATE], dt, kind="ExternalInput")
    bias_d = nc.dram_tensor("bias", [NSTATE, 1], f32, kind="ExternalInput")
    init3_d = nc.dram_tensor("init3", [NSTATE, 96], dt, kind="ExternalInput")
    xT_d = nc.dram_tensor("xT", [xrows, BL], dt, kind="ExternalInput")
    yT_d = nc.dram_tensor("yT", [xrows, BL], dt, kind="ExternalOutput")

    with (
        nc.sbuf_tensor([K, R * BL], dt) as state,
        nc.sbuf_tensor([K, NSTATE], dt) as wT_s,
        nc.sbuf_tensor([NSTATE, 1], f32) as bias_s,
        nc.psum_tensor([NSTATE, 4096], f32) as psum,
        nc.semaphore() as pe_sem,
        nc.semaphore() as act_sem,
        nc.semaphore() as init_sem,
        nc.semaphore() as xe_sem,
        nc.semaphore() as xo_sem,
        nc.semaphore() as oe_sem,
        nc.semaphore() as oo_sem,
        nc.Block() as block,
    ):
        xsems = (xe_sem, xo_sem)
        osems = (oe_sem, oo_sem)

        @block.tensor
        def _(tensor):
            for s in range(s_steps):
                slot = s % R
                bank = s % 8
                # extra waits (rare) go on standalone instructions BEFORE the
                # matmul: each instruction has a single wait slot.
                if s == 0:
                    nc.tensor.wait_ge(init_sem, 48)  # wT + bias + init3 DMAs
                elif s % BLK == 0:
                    g = s // BLK
                    nc.tensor.wait_ge(xsems[g % 2], 16 * (g // 2 + 1))
                mm = nc.tensor.matmul(
                    psum[0:NSTATE, bank * 512 : bank * 512 + BL],
                    wT_s[:, :],
                    state[:, slot * BL : (slot + 1) * BL],
                    start=True,
                    stop=True,
                )
                if s == 0:
                    mm._wait_ge(xe_sem, 16)  # x block 0 staged
                else:
                    mm._wait_ge(act_sem, s)
                mm.then_inc(pe_sem, 1)

        @block.scalar
        def _(scalar):
            Tanh = mybir.ActivationFunctionType.Tanh
            for s in range(s_steps):
                # rows written: step 0 only h0, step 1 h0+h1, then all 33
                nr = 16 if s == 0 else (32 if s == 1 else NSTATE)
                dslot = (s + 1) % R
                bank = s % 8
                if (s + 1) % BLK == 0 and (s + 1) >= R:
                    bi = (s + 1 - R) // BLK  # out block whose slots we start reusing
                    nc.scalar.wait_ge(osems[bi % 2], 16 * (bi // 2 + 1))
                act = nc.scalar.activation(
                    state[0:nr, dslot * BL : dslot * BL + BL],
                    psum[0:nr, bank * 512 : bank * 512 + BL],
                    Tanh,
                    bias=bias_s[0:nr, 0:1],
                )
                act._wait_ge(pe_sem, s + 1)
                act.then_inc(act_sem, 1)

        @block.sync
        def _(sync):
            nc.sync.dma_start(wT_s[:, :], wT_d[:, :]).then_inc(init_sem, 16)
            nc.sync.dma_start(bias_s[:, :], bias_d[:, :]).then_inc(init_sem, 16)
            # v_0/v_1/v_2 initial h values (rows 0..32 of slots 0..2)
            nc.sync.dma_start(state[0:NSTATE, 0:96], init3_d[:, :]).then_inc(
                init_sem, 16
            )

            def stage(g):
                d = nc.sync.dma_start(
                    state[NSTATE : NSTATE + 1, (g % 2) * BLK * BL : ((g % 2) + 1) * BLK * BL],
                    xT_d[g * BLK : (g + 1) * BLK, :],
                )
                if g >= 2:
                    d._wait_ge(pe_sem, min((g - 1) * BLK, s_steps))
                d.then_inc(xsems[g % 2], 16)

            stage(0)
            if nblk > 1:
                stage(1)
            for b in range(nblk):
                od = nc.sync.dma_start(
                    yT_d[b * BLK : (b + 1) * BLK, :],
                    state[NSTATE - 1 : NSTATE, (b % 2) * BLK * BL : ((b % 2) + 1) * BLK * BL],
                )
                od._wait_ge(act_sem, min(b * BLK + BLK - 1, s_steps))
                od.then_inc(osems[b % 2], 16)
                if b + 2 < nblk:
                    stage(b + 2)
            nc.sync.wait_ge(oe_sem, 16 * ((nblk + 1) // 2))
            nc.sync.wait_ge(oo_sem, 16 * (nblk // 2))

    return nc


def _host_prep(inputs, t_len=T, xrows=XROWS, half=False):
    """Build per-core input maps from the full problem inputs."""
    x = np.asarray(inputs["x"], np.float32).reshape(B, t_len)
    W_ih0 = np.asarray(inputs["W_ih0"], np.float32)
    W_hh0 = np.asarray(inputs["W_hh0"], np.float32)
    b_ih0 = np.asarray(inputs["b_ih0"], np.float32)
    b_hh0 = np.asarray(inputs["b_hh0"], np.float32)
    W_ih1 = np.asarray(inputs["W_ih1"], np.float32)
    W_hh1 = np.asarray(inputs["W_hh1"], np.float32)
    b_ih1 = np.asarray(inputs["b_ih1"], np.float32)
    b_hh1 = np.asarray(inputs["b_hh1"], np.float32)
    W_ihp = np.asarray(inputs["W_ihp"], np.float32)
    W_hhp = np.asarray(inputs["W_hhp"], np.float32)
    b_ihp = np.asarray(inputs["b_ihp"], np.float32)
    b_hhp = np.asarray(inputs["b_hhp"], np.float32)
    prev_h0 = np.asarray(inputs["prev_h0"], np.float32)
    post_h0 = np.asarray(inputs["post_h0"], np.float32)

    # xn = 0.5*x + 0.5 folded: matmul x-column carries 0.5*W_ih0, bias +0.5*W_ih0
    wx = 0.5 * W_ih0[:, 0]  # [16]
    M = np.zeros((NSTATE, K), np.float32)
    M[0:16, 0:16] = W_hh0
    M[0:16, 33] = wx
    M[16:32, 0:16] = W_ih1
    M[16:32, 16:32] = W_hh1
    M[32, 16:32] = W_ihp[0, :]
    M[32, 32] = W_hhp[0, 0]
    wT = np.ascontiguousarray(M.T)  # [34, 33]

    bias = np.zeros((NSTATE, 1), np.float32)
    bias[0:16, 0] = b_ih0 + b_hh0 + wx
    bias[16:32, 0] = b_ih1 + b_hh1
    bias[32, 0] = b_ihp[0] + b_hhp[0]

    v0 = np.zeros((NSTATE,), np.float32)
    v0[0:16] = prev_h0[0]
    v0[16:32] = prev_h0[1]
    v0[32] = post_h0[0, 0]
    init3 = np.ascontiguousarray(
        np.repeat(v0[:, None], 96, axis=1)
    )  # slots 0..2, all 32 cols

    dt = np.float16 if half else np.float32
    in_maps = []
    for c in range(NCORES):
        xs = x[c * BL : (c + 1) * BL, :]  # [32, t_len]
        xT = np.zeros((xrows, BL), dt)
        xT[0:t_len, :] = xs.T
        in_maps.append(
            {"wT": wT.astype(dt), "bias": bias, "init3": init3.astype(dt), "xT": xT}
        )
    return in_maps


HALF = True  # fp16 state/weights: faster PE streaming, ~2e-4 rel error


def kernel(**inputs) -> np.ndarray:
    from concourse.bass_utils import run_bass_kernel_spmd

    if "nc" not in _CACHE:
        _CACHE["nc"] = _build_nc(half=HALF)
    nc = _CACHE["nc"]

    in_maps = _host_prep(inputs, half=HALF)
    res = run_bass_kernel_spmd(nc, in_maps, core_ids=list(range(NCORES)))

    y = np.empty((B, T, OUT), np.float32)
    for c in range(NCORES):
        yT = res.results[c]["yT"]  # [XROWS, 32]; y[t] = row t+3
        y[c * BL : (c + 1) * BL, :, 0] = yT[3 : 3 + T, :].T.astype(np.float32)
    return y


# revision 14
# speedup vs baseline: 1.0920x; 1.0920x over previous
"""Trainium2 Bass kernel for a 3-layer tanh RNN (B=256, T=16384, H=16).

Strategy
--------
Data-parallel: batch 256 is split across 8 NeuronCores (32 per core). The
three RNN layers are fused into ONE recurrence via pipeline skew: at
iteration s the combined state vector is

    v_s = [h0 (16 rows); h1 (16 rows); hp (1 row); x_s (1 row)]   (34 rows)

laid out h-on-partitions, batch-on-free-dim [34, 32]. One 34x33 matmul
computes all pre-activations (layer0 from h0+x, layer1 from h0+h1, layerp
from h1+hp -- using values one step stale, which is exactly the layer skew),
then one scalar-engine tanh with per-partition bias produces v_{s+1}
rows 0..32.  Row 33 (raw x; min-max normalization is folded into the
matmul column and bias) is staged ahead by DMA from a host-transposed
x.  Outputs y[t] = row 32 of v_{t+3}; they are DMA'd out in 256-state
blocks from the rotating slot buffer.

Per step the critical path is exactly: matmul -> tanh -> matmul ...
(one TensorE instruction + one ScalarE instruction, with waits attached
directly to the instructions).
"""

import sys

sys.path.insert(0, "/opt/trn_rl_repo")

import numpy as np

# ---- problem constants (hardcoded; kernel.py must be self-contained) ----
B, T, IN, H, OUT = 256, 16384, 1, 16, 1
NCORES = 8
BL = B // NCORES  # 32 batch elements per core
NSTATE = 2 * H + OUT  # 33 state rows produced per step
K = NSTATE + 1  # 34 = contraction dim (state + x row)
S = T + 2  # 16386 iterations (pipeline skew of 2)
R = 512  # rotating state slots in SBUF
BLK = 256  # states per staging/output block
NBLK = (S + 1 + BLK - 1) // BLK  # 65 blocks cover states v_0..v_{NBLK*BLK-1}
XROWS = NBLK * BLK  # 16640 padded x/y rows

_CACHE = {}


def _build_nc(s_steps=S, nblk=NBLK, half=False):
    import concourse.bass as bass
    import concourse.mybir as mybir

    f32 = mybir.dt.float32
    dt = mybir.dt.float16 if half else f32
    xrows = nblk * BLK

    nc = bass.Bass()
    wT_d = nc.dram_tensor("wT", [K, NSTATE], dt, kind="ExternalInput")
    bias_d = nc.dram_tensor("bias", [NSTATE, 1], f32, kind="ExternalInput")
    init3_d = nc.dram_tensor("init3", [NSTATE, 96], dt, kind="ExternalInput")
    xT_d = nc.dram_tensor("xT", [xrows, BL], dt, kind="ExternalInput")
    yT_d = nc.dram_tensor("yT", [xrows, BL], dt, kind="ExternalOutput")

    with (
        nc.sbuf_tensor([K, R * BL], dt) as state,
        nc.sbuf_tensor([K, NSTATE], dt) as wT_s,
        nc.sbuf_tensor([NSTATE, 1], f32) as bias_s,
        nc.psum_tensor([NSTATE, 4096], f32) as psum,
        nc.semaphore() as pe_sem,
        nc.semaphore() as act_sem,
        nc.semaphore() as init_sem,
        nc.semaphore() as xe_sem,
        nc.semaphore() as xo_sem,
        nc.semaphore() as oe_sem,
        nc.semaphore() as oo_sem,
        nc.Block() as block,
    ):
        xsems = (xe_sem, xo_sem)
        osems = (oe_sem, oo_sem)

        @block.tensor
        def _(tensor):
            for s in range(s_steps):
                slot = s % R
                bank = s % 8
                # extra waits (rare) go on standalone instructions BEFORE the
                # matmul: each instruction has a single wait slot.
                if s == 0:
                    nc.tensor.wait_ge(init_sem, 48)  # wT + bias + init3 DMAs
                elif s % BLK == 0:
                    g = s // BLK
                    nc.tensor.wait_ge(xsems[g % 2], 16 * (g // 2 + 1))
                mm = nc.tensor.matmul(
                    psum[0:NSTATE, bank * 512 : bank * 512 + BL],
                    wT_s[:, :],
                    state[:, slot * BL : (slot + 1) * BL],
                    start=True,
                    stop=True,
                )
                if s == 0:
                    mm._wait_ge(xe_sem, 16)  # x block 0 staged
                else:
                    mm._wait_ge(act_sem, s)
                mm.then_inc(pe_sem, 1)
                # Keep-warm dummy matmul: the chain matmul alone is ~5% PE
                # duty, so the HAM clock gate would hold the PE at 1.2 GHz;
                # one extra matmul per step keeps it at 2.4 GHz (measured
                # 571 -> 523 ns/step).  Bank (s+4)%8 cols 480.. is never
                # read by the concurrently running ACT steps (they read
                # banks s%8/(s+1)%8, cols < 32 only).
                db = (s + 4) % 8
                nc.tensor.matmul(
                    psum[0:32, db * 512 + 480 : db * 512 + 512],
                    wT_s[:, 0:32],
                    wT_s[:, 0:32],
                    start=True,
                    stop=True,
                )

        @block.scalar
        def _(scalar):
            Tanh = mybir.ActivationFunctionType.Tanh
            for s in range(s_steps):
                # rows written: step 0 only h0, step 1 h0+h1, then all 33
                nr = 16 if s == 0 else (32 if s == 1 else NSTATE)
                dslot = (s + 1) % R
                bank = s % 8
                if (s + 1) % BLK == 0 and (s + 1) >= R:
                    bi = (s + 1 - R) // BLK  # out block whose slots we start reusing
                    nc.scalar.wait_ge(osems[bi % 2], 16 * (bi // 2 + 1))
                act = nc.scalar.activation(
                    state[0:nr, dslot * BL : dslot * BL + BL],
                    psum[0:nr, bank * 512 : bank * 512 + BL],
                    Tanh,
                    bias=bias_s[0:nr, 0:1],
                )
                act._wait_ge(pe_sem, s + 1)
                act.then_inc(act_sem, 1)

        @block.sync
        def _(sync):
            nc.sync.dma_start(wT_s[:, :], wT_d[:, :]).then_inc(init_sem, 16)
            nc.sync.dma_start(bias_s[:, :], bias_d[:, :]).then_inc(init_sem, 16)
            # v_0/v_1/v_2 initial h values (rows 0..32 of slots 0..2)
            nc.sync.dma_start(state[0:NSTATE, 0:96], init3_d[:, :]).then_inc(
                init_sem, 16
            )

            def stage(g):
                d = nc.sync.dma_start(
                    state[NSTATE : NSTATE + 1, (g % 2) * BLK * BL : ((g % 2) + 1) * BLK * BL],
                    xT_d[g * BLK : (g + 1) * BLK, :],
                )
                if g >= 2:
                    d._wait_ge(pe_sem, min((g - 1) * BLK, s_steps))
                d.then_inc(xsems[g % 2], 16)

            stage(0)
            if nblk > 1:
                stage(1)
            for b in range(nblk):
                od = nc.sync.dma_start(
                    yT_d[b * BLK : (b + 1) * BLK, :],
                    state[NSTATE - 1 : NSTATE, (b % 2) * BLK * BL : ((b % 2) + 1) * BLK * BL],
                )
                od._wait_ge(act_sem, min(b * BLK + BLK - 1, s_steps))
                od.then_inc(osems[b % 2], 16)
                if b + 2 < nblk:
                    stage(b + 2)
            nc.sync.wait_ge(oe_sem, 16 * ((nblk + 1) // 2))
            nc.sync.wait_ge(oo_sem, 16 * (nblk // 2))

    return nc


def _host_prep(inputs, t_len=T, xrows=XROWS, half=False):
    """Build per-core input maps from the full problem inputs."""
    x = np.asarray(inputs["x"], np.float32).reshape(B, t_len)
    W_ih0 = np.asarray(inputs["W_ih0"], np.float32)
    W_hh0 = np.asarray(inputs["W_hh0"], np.float32)
    b_ih0 = np.asarray(inputs["b_ih0"], np.float32)
    b_hh0 = np.asarray(inputs["b_hh0"], np.float32)
    W_ih1 = np.asarray(inputs["W_ih1"], np.float32)
    W_hh1 = np.asarray(inputs["W_hh1"], np.float32)
    b_ih1 = np.asarray(inputs["b_ih1"], np.float32)
    b_hh1 = np.asarray(inputs["b_hh1"], np.float32)
    W_ihp = np.asarray(inputs["W_ihp"], np.float32)
    W_hhp = np.asarray(inputs["W_hhp"], np.float32)
    b_ihp = np.asarray(inputs["b_ihp"], np.float32)
    b_hhp = np.asarray(inputs["b_hhp"], np.float32)
    prev_h0 = np.asarray(inputs["prev_h0"], np.float32)
    post_h0 = np.asarray(inputs["post_h0"], np.float32)

    # xn = 0.5*x + 0.5 folded: matmul x-column carries 0.5*W_ih0, bias +0.5*W_ih0
    wx = 0.5 * W_ih0[:, 0]  # [16]
    M = np.zeros((NSTATE, K), np.float32)
    M[0:16, 0:16] = W_hh0
    M[0:16, 33] = wx
    M[16:32, 0:16] = W_ih1
    M[16:32, 16:32] = W_hh1
    M[32, 16:32] = W_ihp[0, :]
    M[32, 32] = W_hhp[0, 0]
    wT = np.ascontiguousarray(M.T)  # [34, 33]

    bias = np.zeros((NSTATE, 1), np.float32)
    bias[0:16, 0] = b_ih0 + b_hh0 + wx
    bias[16:32, 0] = b_ih1 + b_hh1
    bias[32, 0] = b_ihp[0] + b_hhp[0]

    v0 = np.zeros((NSTATE,), np.float32)
    v0[0:16] = prev_h0[0]
    v0[16:32] = prev_h0[1]
    v0[32] = post_h0[0, 0]
    init3 = np.ascontiguousarray(
        np.repeat(v0[:, None], 96, axis=1)
    )  # slots 0..2, all 32 cols

    dt = np.float16 if half else np.float32
    in_maps = []
    for c in range(NCORES):
        xs = x[c * BL : (c + 1) * BL, :]  # [32, t_len]
        xT = np.zeros((xrows, BL), dt)
        xT[0:t_len, :] = xs.T
        in_maps.append(
            {"wT": wT.astype(dt), "bias": bias, "init3": init3.astype(dt), "xT": xT}
        )
    return in_maps


HALF = True  # fp16 state/weights: faster PE streaming, ~2e-4 rel error


def kernel(**inputs) -> np.ndarray:
    from concourse.bass_utils import run_bass_kernel_spmd

    if "nc" not in _CACHE:
        _CACHE["nc"] = _build_nc(half=HALF)
    nc = _CACHE["nc"]

    in_maps = _host_prep(inputs, half=HALF)
    res = run_bass_kernel_spmd(nc, in_maps, core_ids=list(range(NCORES)))

    y = np.empty((B, T, OUT), np.float32)
    for c in range(NCORES):
        yT = res.results[c]["yT"]  # [XROWS, 32]; y[t] = row t+3
        y[c * BL : (c + 1) * BL, :, 0] = yT[3 : 3 + T, :].T.astype(np.float32)
    return y
